# revision 25
# baseline (speedup 1.0000x reference)
"""Trainium2 Bass kernel for the MembraneLayer problem.

Computation (per batch element b, per output neuron o):
    h[b, t, :] = inputs[b, t, :] @ w                       # (T, O)
    syn[b, 0] = mem[b, 0] = 0
    syn[b, t+1] = alpha * syn[b, t] + h[b, t]              # t = 0..T-2
    mem[b, t+1] = beta  * mem[b, t] + (1-beta) * syn[b, t]
Returns (syn_rec, mem_rec), each (B, T, O) float32.

Mapping: data-parallel over batch across 8 NeuronCores (16 batch rows per
core).  The host marshals inputs c-major as bf16 (hi, lo) pairs --
x = hi + lo to ~2^-17 relative -- and outputs to (O, BS, T), so every DMA
is a large fully-contiguous transfer and the matmul runs at full bf16 PE
rate: h = x_hi@w_hi + x_hi@w_lo + x_lo@w_hi accumulated in fp32 PSUM
(the dropped lo*lo term is ~2^-18).  Three matmuls per contraction block
replace one quarter-rate fp32 matmul (25% fewer PE cycles, and the DMA
stream, not the PE, becomes the bottleneck).  The two first-order
recurrences run as DVE tensor_tensor_scan instructions (state =
data0*state + data1 along the free axis, one recurrence per partition,
fp32 state).  The (1-beta)*syn cross term runs on the scalar engine.
"""

import numpy as np
import ml_dtypes
from contextlib import ExitStack

import concourse.bacc as bacc
import concourse.bass as bass
import concourse.tile as tile
import concourse.mybir as mybir
from concourse import bass_utils

B, T, I, O = 128, 512, 700, 128
NCORES = 8
BS = B // NCORES            # batch rows per core (16)
G = 4                       # batch rows per quad (tile group)
NQ = BS // G                # quads per core (4)
KFULL = 5                   # full 128-row contraction blocks
KREM = I - KFULL * 128      # 60 remaining contraction rows
F32 = mybir.dt.float32
BF16 = mybir.dt.bfloat16
NPBF16 = ml_dtypes.bfloat16

_CACHE = {}


def _build_nc():
    nc = bacc.Bacc("TRN2", target_bir_lowering=False, debug=False)

    # Host-marshalled: x (c-major, (I, BS, T)) split into bf16 hi/lo.
    xh_d = nc.dram_tensor("x_hi", [I, BS, T], BF16, kind="ExternalInput")
    xl_d = nc.dram_tensor("x_lo", [I, BS, T], BF16, kind="ExternalInput")
    wh_d = nc.dram_tensor("w_hi", [I, O], BF16, kind="ExternalInput")
    wl_d = nc.dram_tensor("w_lo", [I, O], BF16, kind="ExternalInput")
    a_bc_d = nc.dram_tensor("alpha_bc", [O, T], F32, kind="ExternalInput")
    b_bc_d = nc.dram_tensor("beta_bc", [O, T], F32, kind="ExternalInput")
    omb_d = nc.dram_tensor("omb", [O, 1], F32, kind="ExternalInput")
    # Outputs in (O, BS, T); host transposes back to (BS, T, O).
    syn_d = nc.dram_tensor("syn", [O, BS, T], F32, kind="ExternalOutput")
    mem_d = nc.dram_tensor("mem", [O, BS, T], F32, kind="ExternalOutput")

    mult = mybir.AluOpType.mult
    add = mybir.AluOpType.add

    with tile.TileContext(nc) as tc, ExitStack() as ctx:
        const_pool = ctx.enter_context(tc.tile_pool(name="const", bufs=1))
        x_pool = ctx.enter_context(tc.tile_pool(name="xin", bufs=10))
        x0_pool = ctx.enter_context(tc.tile_pool(name="x0", bufs=10))
        x0r_pool = ctx.enter_context(tc.tile_pool(name="x0r", bufs=2))
        xr_pool = ctx.enter_context(tc.tile_pool(name="xrem", bufs=4))
        psum_pool = ctx.enter_context(
            tc.tile_pool(name="hpsum", bufs=8, space=bass.MemorySpace.PSUM)
        )
        syn_pool = ctx.enter_context(tc.tile_pool(name="synout", bufs=6))
        mem_pool = ctx.enter_context(tc.tile_pool(name="memout", bufs=6))
        u_pool = ctx.enter_context(tc.tile_pool(name="u", bufs=3))

        # --- PE warm-up (bf16, ~60ns/matmul) ---
        # The PE starts HAM-throttled (1.2 GHz) and needs ~3.4us of activity
        # to unthrottle; keep it continuously busy from kernel start until
        # the first real operand lands (~12us).
        warm_sb = const_pool.tile([128, 128], BF16)
        nc.gpsimd.memset(warm_sb[:, :], 0.0)
        warm_ps = psum_pool.tile([128, 64], F32, tag="ps")
        for _ in range(140):
            nc.tensor.matmul(
                warm_ps[:, :],
                warm_sb[:, :],
                warm_sb[:, 0:64],
                start=True,
                stop=True,
            )

        # --- constants (SWDGE ring, so x loads lead the HWDGE rings) ---
        # w*_sb[p, k*O + o] = w[128k + p, o]   (contraction on partitions)
        w_sb = {}
        for nm, dram in [("hi", wh_d), ("lo", wl_d)]:
            wt = const_pool.tile([128, KFULL * O], BF16, tag=f"w_{nm}")
            nc.gpsimd.dma_start(
                wt[:, :].rearrange("p (k o) -> p k o", k=KFULL),
                dram[0 : KFULL * 128, :].rearrange("(k p) o -> p k o", p=128),
            )
            wr = const_pool.tile([KREM, O], BF16, tag=f"wr_{nm}")
            nc.gpsimd.dma_start(wr[:, :], dram[KFULL * 128 : I, :])
            w_sb[nm] = (wt, wr)
        a_bc = const_pool.tile([128, T], F32)
        nc.gpsimd.dma_start(a_bc[:, :], a_bc_d[:, :])
        b_bc = const_pool.tile([128, T], F32)
        nc.gpsimd.dma_start(b_bc[:, :], b_bc_d[:, :])
        omb_sb = const_pool.tile([128, 1], F32)
        nc.gpsimd.dma_start(omb_sb[:, :], omb_d[:, :])

        for q in range(NQ):
            b0 = q * G
            if q == 0:
                # Redundant per-piece loads of just the FIRST batch row (it
                # re-arrives inside the quad tiles below): 128KB pieces land
                # first, so real matmuls start earlier.
                b0_rhs = []
                for k in range(KFULL):
                    xg_h = x0_pool.tile([128, T], BF16, tag="h")
                    nc.sync.dma_start(xg_h[:, :], xh_d[128 * k : 128 * (k + 1), 0, :])
                    xg_l = x0_pool.tile([128, T], BF16, tag="l")
                    nc.scalar.dma_start(
                        xg_l[:, :], xl_d[128 * k : 128 * (k + 1), 0, :]
                    )
                    b0_rhs.append((xg_h[:, :], xg_l[:, :]))
                xgr_h = x0r_pool.tile([KREM, T], BF16, tag="h")
                nc.sync.dma_start(xgr_h[:, :], xh_d[KFULL * 128 : I, 0, :])
                xgr_l = x0r_pool.tile([KREM, T], BF16, tag="l")
                nc.scalar.dma_start(xgr_l[:, :], xl_d[KFULL * 128 : I, 0, :])
                b0_rhs.append((xgr_h[:, :], xgr_l[:, :]))

            # Quad loads: hi on the SP ring, lo on the ACT ring (512KB each).
            xks = []
            for k in range(KFULL):
                xq_h = x_pool.tile([128, G * T], BF16, tag="h")
                nc.sync.dma_start(
                    xq_h[:, :].rearrange("p (g t) -> p g t", g=G),
                    xh_d[128 * k : 128 * (k + 1), b0 : b0 + G, :],
                )
                xq_l = x_pool.tile([128, G * T], BF16, tag="l")
                nc.scalar.dma_start(
                    xq_l[:, :].rearrange("p (g t) -> p g t", g=G),
                    xl_d[128 * k : 128 * (k + 1), b0 : b0 + G, :],
                )
                xks.append((xq_h, xq_l))
            xr_h = xr_pool.tile([KREM, G * T], BF16, tag="h")
            nc.sync.dma_start(
                xr_h[:, :].rearrange("p (g t) -> p g t", g=G),
                xh_d[KFULL * 128 : I, b0 : b0 + G, :],
            )
            xr_l = xr_pool.tile([KREM, G * T], BF16, tag="l")
            nc.scalar.dma_start(
                xr_l[:, :].rearrange("p (g t) -> p g t", g=G),
                xl_d[KFULL * 128 : I, b0 : b0 + G, :],
            )

            def quad_rhs(g, k):
                sl = slice(g * T, (g + 1) * T)
                if k < KFULL:
                    return (xks[k][0][:, sl], xks[k][1][:, sl])
                return (xr_h[:, sl], xr_l[:, sl])

            for g in range(G):
                # h^T for batch row b0+g: ps[o, t] = h[b0+g, t, o]
                # = sum_k (xh*wh + xh*wl + xl*wh)
                ps = psum_pool.tile([128, T], F32, tag="ps")
                first = True
                for k in range(KFULL + 1):
                    if q == 0 and g == 0:
                        rh, rl = b0_rhs[k]
                    else:
                        rh, rl = quad_rhs(g, k)
                    wt, wr = w_sb["hi"]
                    lw_hi = wt[:, k * O : (k + 1) * O] if k < KFULL else wr[:, :]
                    wt, wr = w_sb["lo"]
                    lw_lo = wt[:, k * O : (k + 1) * O] if k < KFULL else wr[:, :]
                    last = k == KFULL
                    nc.tensor.matmul(
                        ps[:, :], lw_hi, rh, start=first, stop=False
                    )
                    first = False
                    nc.tensor.matmul(ps[:, :], lw_lo, rh, start=False, stop=False)
                    nc.tensor.matmul(
                        ps[:, :], lw_hi, rl, start=False, stop=last
                    )

                # syn[:, t+1] = alpha*syn[:, t] + h[:, t]
                syn_t = syn_pool.tile([128, T], F32)
                nc.vector.memset(syn_t[:, 0:1], 0.0)
                nc.vector.tensor_tensor_scan(
                    syn_t[:, 1:T],
                    a_bc[:, 0 : T - 1],
                    ps[:, 0 : T - 1],
                    0.0,
                    mult,
                    add,
                )

                # u[:, t] = (1-beta)*syn[:, t] on the scalar engine
                u = u_pool.tile([128, T - 1], F32)
                nc.scalar.mul(u[:, :], syn_t[:, 0 : T - 1], omb_sb[:, :])

                # mem[:, t+1] = beta*mem[:, t] + u[:, t]
                mem_t = mem_pool.tile([128, T], F32)
                nc.vector.memset(mem_t[:, 0:1], 0.0)
                nc.vector.tensor_tensor_scan(
                    mem_t[:, 1:T],
                    b_bc[:, 0 : T - 1],
                    u[:, :],
                    0.0,
                    mult,
                    add,
                )

                # Store each batch row as soon as its scans finish; the
                # kernel tail then only waits on the last row's chain.
                nc.sync.dma_start(syn_d[:, b0 + g, :], syn_t[:, :])
                nc.scalar.dma_start(mem_d[:, b0 + g, :], mem_t[:, :])

    nc.compile()
    return nc


def get_nc():
    if "nc" not in _CACHE:
        _CACHE["nc"] = _build_nc()
    return _CACHE["nc"]


def make_in_maps(inputs, w, alpha, beta):
    x_t = np.asarray(inputs, dtype=np.float32).transpose(2, 0, 1)  # (I, B, T) view
    x_hi = x_t.astype(NPBF16)
    x_lo = (x_t - x_hi.astype(np.float32)).astype(NPBF16)
    w = np.asarray(w, dtype=np.float32)
    w_hi = w.astype(NPBF16)
    w_lo = (w - w_hi.astype(np.float32)).astype(NPBF16)
    alpha = np.asarray(alpha, dtype=np.float32).reshape(O)
    beta = np.asarray(beta, dtype=np.float32).reshape(O)
    a_bc = np.ascontiguousarray(np.broadcast_to(alpha[:, None], (O, T)))
    b_bc = np.ascontiguousarray(np.broadcast_to(beta[:, None], (O, T)))
    omb = np.ascontiguousarray((1.0 - beta)[:, None])
    return [
        {
            "x_hi": np.ascontiguousarray(x_hi[:, i * BS : (i + 1) * BS, :]),
            "x_lo": np.ascontiguousarray(x_lo[:, i * BS : (i + 1) * BS, :]),
            "w_hi": w_hi,
            "w_lo": w_lo,
            "alpha_bc": a_bc,
            "beta_bc": b_bc,
            "omb": omb,
        }
        for i in range(NCORES)
    ]


def kernel(inputs, w, alpha, beta):
    nc = get_nc()
    in_maps = make_in_maps(inputs, w, alpha, beta)
    res = bass_utils.run_bass_kernel_spmd(nc, in_maps, list(range(NCORES))).results
    # Per-core outputs are (O, BS, T); gather over batch then -> (B, T, O).
    syn = np.concatenate([r["syn"] for r in res], axis=1).transpose(1, 2, 0)
    mem = np.concatenate([r["mem"] for r in res], axis=1).transpose(1, 2, 0)
    return np.ascontiguousarray(syn), np.ascontiguousarray(mem)
